# revision 1
# baseline (speedup 1.0000x reference)
"""Trainium2 Bass kernel for the contrastive loss (nn_Contrast).

loss = LAM * mean_i(-log s_mp[i]) + (1-LAM) * mean_i(-log s_sc[i])
  S = exp(cos(n1_i, n2_j)/tau);  n1 = norm(proj(z_mp)), n2 = norm(proj(z_sc))
  s_mp[i] = sum_j S[i, c_ij] / rowsum_i ;  s_sc[i] = sum_j S[c_ij, i] / colsum_i

Sharding: rows of S across 8 cores (1024 rows each). Each core:
  - projects its z_mp row-block (transposed pipeline, bf16 matmuls)
  - projects the FULL z_sc (redundant; needed as rhs of its S row-block)
  - streams its S block tile-by-tile: exp (with per-row 1/(norm*tau) scale
    folded into the ACT scale), rowsum via ACT accum, colsum + masked
    column-sums via PE ones-matmuls, mp-edge extraction via masked
    tensor_tensor_reduce. Edge masks are built host-side from pos.
  - one 64KB AllReduce combines colsum and the sc-edge numerator partials.
Host combines 8 partial scalars.
"""

import numpy as np
import ml_dtypes

N = 8192
HID = 512
TAU = 0.8
LAM = 0.5
NCORES = 8
B = N // NCORES          # rows per core = 1024
RT = B // 128            # row tiles per core = 8
CC = N // 1024           # 1024-wide col chunks = 8
KT = HID // 128          # contraction tiles = 4

bf16 = ml_dtypes.bfloat16


def _split_multi_waits(nc, mybir):
    """This container's walrus accepts only ONE sync-wait per instruction;
    Tile batches several. Split extras into single-wait NoOps."""
    counter = [0]
    for f in nc.m.functions:
        for bb in f.blocks:
            new_insts = []
            changed = False
            for inst in bb.instructions:
                si = inst.sync_info
                if si is not None and si.on_wait is not None and len(si.on_wait) > 1:
                    waits = list(si.on_wait)
                    for w in waits[:-1]:
                        counter[0] += 1
                        new_insts.append(mybir.InstNoOp(
                            name=f"I-wsplit-{counter[0]}",
                            engine=inst.engine,
                            sync_info=mybir.SyncInfo(on_wait=[w], on_update=[]),
                            bass_nofuse=True,
                        ))
                    inst.sync_info = mybir.SyncInfo(
                        on_wait=[waits[-1]], on_update=list(si.on_update or []))
                    changed = True
                new_insts.append(inst)
            if changed:
                bb.instructions = new_insts
    return nc


def build_program():
    import concourse.bass as bass
    import concourse.mybir as mybir
    import concourse.tile as tile

    dt = mybir.dt
    F32, BF16 = dt.float32, dt.bfloat16
    Act = mybir.ActivationFunctionType
    Alu = mybir.AluOpType

    nc = bass.Bass("TRN2", num_devices=NCORES)

    z_mpt = nc.dram_tensor("z_mpt", [HID, B], BF16, kind="ExternalInput")
    z_sct = nc.dram_tensor("z_sct", [HID, N], BF16, kind="ExternalInput")
    w1t = nc.dram_tensor("w1t", [HID, HID], BF16, kind="ExternalInput")
    w2t = nc.dram_tensor("w2t", [HID, HID], BF16, kind="ExternalInput")
    b1r = nc.dram_tensor("b1r", [1, HID], BF16, kind="ExternalInput")
    b2r = nc.dram_tensor("b2r", [1, HID], BF16, kind="ExternalInput")
    mask_mp = nc.dram_tensor("mask_mp", [CC, RT, 128, 1024], BF16,
                             kind="ExternalInput")
    mask_sc = nc.dram_tensor("mask_sc", [CC, RT, 128, 1024], BF16,
                             kind="ExternalInput")
    out = nc.dram_tensor("out", [1, 2], F32, kind="ExternalOutput")

    rn1_dram = nc.dram_tensor("rn1_dram", [B], F32)
    norm_dram = nc.dram_tensor("norm_dram", [N], F32)
    rn_dram = nc.dram_tensor("rn_dram", [N], F32)
    cc_in = nc.dram_tensor("cc_in", [2, N], F32)
    cc_out = nc.dram_tensor("cc_out", [2, N], F32, addr_space="Shared")

    with tile.TileContext(nc) as tc:
        with tc.tile_pool(name="const", bufs=1) as constp, \
             tc.tile_pool(name="persist", bufs=1) as pers:
            ones_row = constp.tile([1, 1024], BF16, tag="ones_row", name="ones_row")
            nc.vector.memset(ones_row[:], 1.0)
            ones_row_f32 = constp.tile([1, 128], F32, tag="ones_row_f32", name="ones_row_f32")
            nc.vector.memset(ones_row_f32[:], 1.0)
            ones_col = constp.tile([128, 1], BF16, tag="ones_col", name="ones_col")
            nc.vector.memset(ones_col[:], 1.0)
            ones_col_f32 = constp.tile([128, 1], F32, tag="ones_col_f32", name="ones_col_f32")
            nc.vector.memset(ones_col_f32[:], 1.0)

            w1s = [constp.tile([128, HID], BF16, tag=f"w1_{k}", name=f"w1_{k}") for k in range(KT)]
            w2s = [constp.tile([128, HID], BF16, tag=f"w2_{k}", name=f"w2_{k}") for k in range(KT)]
            for k in range(KT):
                nc.sync.dma_start(out=w1s[k][:], in_=w1t[k * 128:(k + 1) * 128, :])
                nc.sync.dma_start(out=w2s[k][:], in_=w2t[k * 128:(k + 1) * 128, :])
            b1s = constp.tile([1, HID], BF16, tag="b1s", name="b1s")
            nc.sync.dma_start(out=b1s[:], in_=b1r[:])
            b2s = constp.tile([1, HID], BF16, tag="b2s", name="b2s")
            nc.sync.dma_start(out=b2s[:], in_=b2r[:])

            # persistent results
            p1T = [pers.tile([128, B], BF16, tag=f"p1T_{k}", name=f"p1T_{k}") for k in range(KT)]
            n2T = [pers.tile([128, N], BF16, tag=f"n2T_{k}", name=f"n2T_{k}") for k in range(KT)]
            scale_mp = pers.tile([128, RT], F32, tag="scale_mp", name="scale_mp")
            rowsum_acc = pers.tile([128, RT * CC], F32, tag="rowsum_acc", name="rowsum_acc")
            nummp_acc = pers.tile([128, RT * CC], F32, tag="nummp_acc", name="nummp_acc")
            out_sb = pers.tile([1, 2], F32, tag="out_sb", name="out_sb")

            # ---------------- Stage A: proj(z_mp block) -> p1T, scale_mp
            with tc.tile_pool(name="stA", bufs=1) as stA, \
                 tc.tile_pool(name="workA", bufs=2) as wkA, \
                 tc.tile_pool(name="psA", bufs=2, space="PSUM") as psA, \
                 tc.tile_pool(name="psA1", bufs=1, space="PSUM") as psA1:
                zmp = [stA.tile([128, B], BF16, tag=f"zmp_{k}", name=f"zmp_{k}") for k in range(KT)]
                for k in range(KT):
                    nc.sync.dma_start(out=zmp[k][:],
                                      in_=z_mpt[k * 128:(k + 1) * 128, :])
                h1 = [stA.tile([128, B], BF16, tag=f"h1_{k}", name=f"h1_{k}") for k in range(KT)]
                for ht in range(KT):
                    hsl = slice(ht * 128, (ht + 1) * 128)
                    ps = psA.tile([128, B], F32, tag="psA", name="psA")
                    for h in range(B // 512):
                        sl = slice(h * 512, (h + 1) * 512)
                        for k in range(KT):
                            nc.tensor.matmul(ps[:, sl], w1s[k][:, hsl],
                                             zmp[k][:, sl],
                                             start=(k == 0), stop=False)
                        nc.tensor.matmul(ps[:, sl], b1s[0:1, hsl],
                                         ones_row[0:1, 0:512],
                                         start=False, stop=True)
                    tmin = wkA.tile([128, B], BF16, tag="tmin", name="tmin")
                    nc.vector.tensor_scalar_min(tmin[:], ps[:], 0.0)
                    texp = wkA.tile([128, B], BF16, tag="texp", name="texp")
                    nc.scalar.activation(texp[:], tmin[:], Act.Exp)
                    nc.vector.scalar_tensor_tensor(h1[ht][:], texp[:], -1.0, ps[:],
                                                   op0=Alu.add, op1=Alu.max)
                norm2h = [psA1.tile([1, 512], F32, tag=f"norm2A_{h}", name=f"norm2A_{h}")
                          for h in range(B // 512)]
                for ht in range(KT):
                    hsl = slice(ht * 128, (ht + 1) * 128)
                    ps2 = psA.tile([128, B], F32, tag="psA", name="psA2")
                    for h in range(B // 512):
                        sl = slice(h * 512, (h + 1) * 512)
                        for k in range(KT):
                            nc.tensor.matmul(ps2[:, sl], w2s[k][:, hsl],
                                             h1[k][:, sl],
                                             start=(k == 0), stop=False)
                        nc.tensor.matmul(ps2[:, sl], b2s[0:1, hsl],
                                         ones_row[0:1, 0:512],
                                         start=False, stop=True)
                    sq = wkA.tile([128, B], BF16, tag="sqA", name="sqA")
                    nc.scalar.activation(sq[:], ps2[:], Act.Square)
                    for h in range(B // 512):
                        sl = slice(h * 512, (h + 1) * 512)
                        nc.tensor.matmul(norm2h[h][0:1, :], ones_col[:], sq[:, sl],
                                         start=(ht == 0), stop=(ht == KT - 1))
                    nc.vector.tensor_copy(p1T[ht][:], ps2[:])
                nrm = wkA.tile([1, B], F32, tag="nrmA", name="nrmA")
                for h in range(B // 512):
                    sl = slice(h * 512, (h + 1) * 512)
                    nc.scalar.activation(nrm[0:1, sl], norm2h[h][:], Act.Sqrt)
                rn1 = wkA.tile([1, B], F32, tag="rn1A", name="rn1A")
                nc.vector.reciprocal(rn1[:], nrm[:])
                nc.vector.tensor_scalar_mul(rn1[:], rn1[:], 1.0 / TAU)
                nc.gpsimd.dma_start(out=rn1_dram[:], in_=rn1[:])
                nc.gpsimd.dma_start(
                    out=scale_mp[:],
                    in_=rn1_dram[:].rearrange("(g p) -> p g", p=128))

            # ---------------- Stage B: proj(full z_sc) -> n2T (normalized)
            with tc.tile_pool(name="h2p", bufs=1) as h2p:
                with tc.tile_pool(name="zscp", bufs=2) as zscp, \
                     tc.tile_pool(name="psB", bufs=2, space="PSUM") as psB, \
                     tc.tile_pool(name="workB", bufs=2) as wkB:
                    h2 = [h2p.tile([128, N], BF16, tag=f"h2_{k}", name=f"h2_{k}")
                          for k in range(KT)]
                    for nch in range(N // 1024):
                        nsl = slice(nch * 1024, (nch + 1) * 1024)
                        zc = [zscp.tile([128, 1024], BF16, tag=f"zc_{k}", name=f"zc_{k}")
                              for k in range(KT)]
                        for k in range(KT):
                            nc.sync.dma_start(out=zc[k][:],
                                              in_=z_sct[k * 128:(k + 1) * 128, nsl])
                        for ht in range(KT):
                            hsl = slice(ht * 128, (ht + 1) * 128)
                            ps = psB.tile([128, 1024], F32, tag="psB", name="psB")
                            for h in range(2):
                                psl = slice(h * 512, (h + 1) * 512)
                                for k in range(KT):
                                    nc.tensor.matmul(ps[:, psl], w1s[k][:, hsl],
                                                     zc[k][:, psl],
                                                     start=(k == 0), stop=False)
                                nc.tensor.matmul(ps[:, psl], b1s[0:1, hsl],
                                                 ones_row[0:1, 0:512],
                                                 start=False, stop=True)
                            tmin = wkB.tile([128, 1024], BF16, tag="tminB", name="tminB")
                            nc.vector.tensor_scalar_min(tmin[:], ps[:], 0.0)
                            texp = wkB.tile([128, 1024], BF16, tag="texpB", name="texpB")
                            nc.scalar.activation(texp[:], tmin[:], Act.Exp)
                            nc.vector.scalar_tensor_tensor(
                                h2[ht][:, nsl], texp[:], -1.0, ps[:],
                                op0=Alu.add, op1=Alu.max)

                # layer 2: unscaled p2T -> n2T tiles; norms accumulated to DRAM
                with tc.tile_pool(name="psB2", bufs=1, space="PSUM") as psB2, \
                     tc.tile_pool(name="psB2n", bufs=2, space="PSUM") as psB2n, \
                     tc.tile_pool(name="workB2", bufs=3) as wkB2:
                    for nch in range(N // 512):
                        nsl = slice(nch * 512, (nch + 1) * 512)
                        pst = [psB2.tile([128, 512], F32, tag=f"pstB2_{ht}", name=f"pstB2_{ht}")
                               for ht in range(KT)]
                        norm2 = psB2n.tile([1, 512], F32, tag="norm2B", name="norm2B")
                        for ht in range(KT):
                            hsl = slice(ht * 128, (ht + 1) * 128)
                            for k in range(KT):
                                nc.tensor.matmul(pst[ht][:], w2s[k][:, hsl],
                                                 h2[k][:, nsl],
                                                 start=(k == 0), stop=False)
                            nc.tensor.matmul(pst[ht][:], b2s[0:1, hsl],
                                             ones_row[0:1, 0:512],
                                             start=False, stop=True)
                            sq = wkB2.tile([128, 512], BF16, tag="sqB", name="sqB")
                            nc.scalar.activation(sq[:], pst[ht][:], Act.Square)
                            nc.tensor.matmul(norm2[0:1, :], ones_col[:], sq[:],
                                             start=(ht == 0), stop=(ht == KT - 1))
                            nc.vector.tensor_copy(n2T[ht][:, nsl], pst[ht][:])
                        nb2 = wkB2.tile([1, 512], F32, tag="nb2", name="nb2")
                        nc.scalar.copy(nb2[:], norm2[:])
                        nc.sync.dma_start(out=norm_dram[nch * 512:(nch + 1) * 512],
                                          in_=nb2[:])
                    # batch rsqrt in [128, 64] layout, back to a row via DRAM
                    nt = wkB2.tile([128, 64], F32, tag="ntB", name="ntB")
                    nc.sync.dma_start(
                        out=nt[:], in_=norm_dram[:].rearrange("(p f) -> p f", p=128))
                    nrt_ = wkB2.tile([128, 64], F32, tag="nrtB", name="nrtB")
                    nc.scalar.activation(nrt_[:], nt[:], Act.Sqrt)
                    rnt = wkB2.tile([128, 64], F32, tag="rntB", name="rntB")
                    nc.vector.reciprocal(rnt[:], nrt_[:])
                    nc.sync.dma_start(out=rn_dram[:].rearrange("(p f) -> p f", p=128),
                                      in_=rnt[:])
                    # scale n2T columns in place, 512 at a time
                    for nch in range(N // 512):
                        nsl = slice(nch * 512, (nch + 1) * 512)
                        rn2 = wkB2.tile([1, 512], F32, tag="rn2B", name="rn2B")
                        nc.sync.dma_start(out=rn2[:],
                                          in_=rn_dram[nch * 512:(nch + 1) * 512])
                        repl = psB2n.tile([128, 512], F32, tag="replB", name="replB")
                        nc.tensor.matmul(repl[:], ones_row_f32[:], rn2[:],
                                         start=True, stop=True)
                        repl_sb = wkB2.tile([128, 512], BF16, tag="replsbB", name="replsbB")
                        nc.scalar.copy(repl_sb[:], repl[:])
                        for ht in range(KT):
                            nc.vector.tensor_tensor(n2T[ht][:, nsl], n2T[ht][:, nsl],
                                                    repl_sb[:], op=Alu.mult)

            # ---------------- Stage C: S block sweep
            with tc.tile_pool(name="workC", bufs=3) as wkC, \
                 tc.tile_pool(name="maskC", bufs=3) as mkC, \
                 tc.tile_pool(name="psC", bufs=2, space="PSUM") as psC, \
                 tc.tile_pool(name="psCa", bufs=1, space="PSUM") as psCa:
                for cc in range(CC):
                    csum = [psCa.tile([1, 512], F32, tag=f"csum_{h}", name=f"csum_{h}")
                            for h in range(2)]
                    nsum = [psCa.tile([1, 512], F32, tag=f"nsum_{h}", name=f"nsum_{h}")
                            for h in range(2)]
                    for rt in range(RT):
                        rsl = slice(rt * 128, (rt + 1) * 128)
                        sp = psC.tile([128, 1024], F32, tag="spC", name="spC")
                        for k in range(KT):
                            for h in range(2):
                                sl = slice(cc * 1024 + h * 512,
                                           cc * 1024 + (h + 1) * 512)
                                psl = slice(h * 512, (h + 1) * 512)
                                nc.tensor.matmul(sp[:, psl], p1T[k][:, rsl],
                                                 n2T[k][:, sl],
                                                 start=(k == 0),
                                                 stop=(k == KT - 1))
                        s_sb = wkC.tile([128, 1024], BF16, tag="s_sb", name="s_sb")
                        idx = rt * CC + cc
                        nc.scalar.activation(s_sb[:], sp[:], Act.Exp,
                                             scale=scale_mp[:, rt:rt + 1],
                                             accum_out=rowsum_acc[:, idx:idx + 1])
                        mmp = mkC.tile([128, 1024], BF16, tag="mmp", name="mmp")
                        nc.sync.dma_start(out=mmp[:], in_=mask_mp[cc, rt])
                        msc = mkC.tile([128, 1024], BF16, tag="msc", name="msc")
                        nc.sync.dma_start(out=msc[:], in_=mask_sc[cc, rt])
                        ttro = wkC.tile([128, 1024], BF16, tag="ttro", name="ttro")
                        nc.vector.scalar_tensor_tensor(
                            ttro[:], s_sb[:], 1.0, mmp[:],
                            op0=Alu.mult, op1=Alu.mult,
                            accum_out=nummp_acc[:, idx:idx + 1])
                        msk = wkC.tile([128, 1024], BF16, tag="msk", name="msk")
                        nc.vector.tensor_tensor(msk[:], s_sb[:], msc[:],
                                                op=Alu.mult)
                        for h in range(2):
                            psl = slice(h * 512, (h + 1) * 512)
                            nc.tensor.matmul(csum[h][0:1, :], ones_col[:],
                                             s_sb[:, psl],
                                             start=(rt == 0), stop=(rt == RT - 1))
                            nc.tensor.matmul(nsum[h][0:1, :], ones_col[:],
                                             msk[:, psl],
                                             start=(rt == 0), stop=(rt == RT - 1))
                    for h in range(2):
                        lo = cc * 1024 + h * 512
                        cb = wkC.tile([1, 512], F32, tag="cb", name="cb")
                        nc.scalar.copy(cb[:], csum[h][:])
                        nc.sync.dma_start(out=cc_in[0, lo:lo + 512], in_=cb[:])
                        nb = wkC.tile([1, 512], F32, tag="nb", name="nb")
                        nc.scalar.copy(nb[:], nsum[h][:])
                        nc.sync.dma_start(out=cc_in[1, lo:lo + 512], in_=nb[:])

            # ---------------- Stage D: combine
            with tc.tile_pool(name="workD", bufs=1) as wkD, \
                 tc.tile_pool(name="psD", bufs=2, space="PSUM") as psD:
                # collective on [colsum ; numsc]
                nc.gpsimd.collective_compute(
                    "AllReduce", Alu.add,
                    replica_groups=[list(range(NCORES))],
                    ins=[cc_in[:]], outs=[cc_out[:]])

                # mp partial: sum_i log(rowsum_i / nummp_i) over my rows
                rowsum_t = wkD.tile([128, RT], F32, tag="rowsum_t", name="rowsum_t")
                nummp_t = wkD.tile([128, RT], F32, tag="nummp_t", name="nummp_t")
                for rt in range(RT):
                    nc.vector.reduce_sum(
                        rowsum_t[:, rt:rt + 1],
                        rowsum_acc[:, rt * CC:(rt + 1) * CC],
                        axis=mybir.AxisListType.X)
                    nc.vector.reduce_sum(
                        nummp_t[:, rt:rt + 1],
                        nummp_acc[:, rt * CC:(rt + 1) * CC],
                        axis=mybir.AxisListType.X)
                recm = wkD.tile([128, RT], F32, tag="recm", name="recm")
                nc.vector.reciprocal(recm[:], nummp_t[:])
                ratm = wkD.tile([128, RT], F32, tag="ratm", name="ratm")
                nc.vector.tensor_tensor(ratm[:], rowsum_t[:], recm[:], op=Alu.mult)
                lnm = wkD.tile([128, RT], F32, tag="lnm", name="lnm")
                lsum_mp = wkD.tile([128, 1], F32, tag="lsum_mp", name="lsum_mp")
                nc.scalar.activation(lnm[:], ratm[:], Act.Ln, accum_out=lsum_mp[:])
                pmp = psD.tile([1, 1], F32, tag="pmp", name="pmp")
                nc.tensor.matmul(pmp[:], lsum_mp[:], ones_col_f32[:],
                                 start=True, stop=True)
                nc.scalar.copy(out_sb[0:1, 0:1], pmp[:])

                # sc full: sum_r log(colsum_r / numsc_r) (same on all cores)
                colsum_t = wkD.tile([128, 64], F32, tag="colsum_t", name="colsum_t")
                nc.sync.dma_start(out=colsum_t[:], in_=cc_out[0].rearrange("(p f) -> p f", p=128))
                numsc_t = wkD.tile([128, 64], F32, tag="numsc_t", name="numsc_t")
                nc.sync.dma_start(out=numsc_t[:], in_=cc_out[1].rearrange("(p f) -> p f", p=128))
                recs = wkD.tile([128, 64], F32, tag="recs", name="recs")
                nc.vector.reciprocal(recs[:], numsc_t[:])
                rats = wkD.tile([128, 64], F32, tag="rats", name="rats")
                nc.vector.tensor_tensor(rats[:], colsum_t[:], recs[:], op=Alu.mult)
                lns = wkD.tile([128, 64], F32, tag="lns", name="lns")
                lsum_sc = wkD.tile([128, 1], F32, tag="lsum_sc", name="lsum_sc")
                nc.scalar.activation(lns[:], rats[:], Act.Ln, accum_out=lsum_sc[:])
                psc = psD.tile([1, 1], F32, tag="psc", name="psc")
                nc.tensor.matmul(psc[:], lsum_sc[:], ones_col_f32[:],
                                 start=True, stop=True)
                nc.scalar.copy(out_sb[0:1, 1:2], psc[:])

                nc.sync.dma_start(out=out[:], in_=out_sb[:])

    _split_multi_waits(nc, mybir)
    return nc


def make_in_maps(z_mp, z_sc, W1, b1, W2, b2, pos):
    z_mp = np.asarray(z_mp, dtype=np.float32)
    z_sc = np.asarray(z_sc, dtype=np.float32)
    W1 = np.asarray(W1, dtype=np.float32)
    W2 = np.asarray(W2, dtype=np.float32)
    b1 = np.asarray(b1, dtype=np.float32)
    b2 = np.asarray(b2, dtype=np.float32)
    r = np.asarray(pos[0]).astype(np.int64)
    c = np.asarray(pos[1]).astype(np.int64)

    z_sct = np.ascontiguousarray(z_sc.T).astype(bf16)
    w1t = np.ascontiguousarray(W1.T).astype(bf16)
    w2t = np.ascontiguousarray(W2.T).astype(bf16)
    b1r = b1.reshape(1, HID).astype(bf16)
    b2r = b2.reshape(1, HID).astype(bf16)

    in_maps = []
    for k in range(NCORES):
        rows = slice(k * B, (k + 1) * B)
        z_mpt = np.ascontiguousarray(z_mp[rows].T).astype(bf16)

        m = np.zeros((B, N), dtype=np.float32)
        sel = (r >= k * B) & (r < (k + 1) * B)
        np.add.at(m, (r[sel] - k * B, c[sel]), 1.0)
        mask_mp = np.ascontiguousarray(
            m.reshape(RT, 128, CC, 1024).transpose(2, 0, 1, 3)).astype(bf16)

        m2 = np.zeros((B, N), dtype=np.float32)
        sel2 = (c >= k * B) & (c < (k + 1) * B)
        np.add.at(m2, (c[sel2] - k * B, r[sel2]), 1.0)
        mask_sc = np.ascontiguousarray(
            m2.reshape(RT, 128, CC, 1024).transpose(2, 0, 1, 3)).astype(bf16)

        in_maps.append({
            "z_mpt": z_mpt, "z_sct": z_sct,
            "w1t": w1t, "w2t": w2t, "b1r": b1r, "b2r": b2r,
            "mask_mp": mask_mp, "mask_sc": mask_sc,
        })
    return in_maps


def combine_outputs(results):
    mp_sum = sum(float(res["out"][0, 0]) for res in results)
    sc_sum = float(results[0]["out"][0, 1])
    loss = (LAM * mp_sum + (1.0 - LAM) * sc_sum) / N
    return np.float32(loss)


def kernel(z_mp, z_sc, W1, b1, W2, b2, pos):
    from concourse.bass_utils import run_bass_kernel_spmd
    nc = build_program()
    in_maps = make_in_maps(z_mp, z_sc, W1, b1, W2, b2, pos)
    res = run_bass_kernel_spmd(nc, in_maps, list(range(NCORES)), trace=False)
    return combine_outputs(res.results)

